# revision 1
# baseline (speedup 1.0000x reference)
"""Trainium2 Bass kernel for nn_Attend_62534723830373.

Reference computation (note: q is UNUSED by the reference):
    scores = einsum('bhid,bhjd->bhij', k, v) * (1/sqrt(128))
    scores = causal_mask(scores)            # strictly-upper masked
    attn   = softmax(scores, axis=-1)
    out    = einsum('bhij,bhjd->bhid', attn, v)

Shapes: [b=2, h=16, s=2048, d=128] fp32. b*h = 32 head-slices sharded
4-per-core across 8 NeuronCores (data/head parallel, no collectives).

Per-head dataflow on one core (matmul chain in bf16, fp32 accumulate):
  - SWDGE cast-load K, V (fp32 HBM -> bf16 SBUF, natural layout), first
    4 row-blocks in their own chunk so compute starts early (chunk-0's
    transposes are emitted before the big load group because DMA
    consumers wait on a coarse per-queue drain point).
  - K^T / V^T built just-in-time per chunk on the PE (transpose-mode
    matmuls); each matrix's 4 blocks land in one 512-wide PSUM tile so
    a single DVE copy moves them to SBUF. The next chunk's transposes
    are emitted at the current chunk's MM2 flush point, so the DVE
    copies queue BEHIND the current chunk's exps.
  - [V | 1] (130-wide, col 129 pad for 4-byte alignment) built with one
    DVE copy + ones memset per head.
  - Per i-chunk (512 wide), j-block pairs share one 1024-wide (2-bank)
    PSUM score tile and ONE exp instruction, emitted with one-pair
    lookahead so the PE always has score matmuls in flight:
      S^T[j, i] = (VT_blk).T @ KT_slice        (PE, contraction d)
      diag blocks: += -2000 strict-lower const (PE matmul w/ identity;
        replaces the DVE tri-mask multiplies; exp of masked -> exact 0)
      E = exp(SCALE * S^T)                     (ACT *or* DVE, see below)
      psum_o[i-blk] += E_slice.T @ [V_blk | 1] (PE, contraction j)
    The ones column makes column 128 of each accumulator the softmax
    denominator.
  - exp is load-balanced between the Scalar engine (real ACT exp) and
    the Vector engine. The DVE path computes exp with a Schraudolph
    bit trick: uint16(round(s*A + B)) bit-cast as bf16 equals
    2^(s*SCALE*log2e) within ~2% rms (A = SCALE*128*log2e, B tuned);
    uint16 saturation at 0 turns masked (-2000-biased) scores into
    bf16 +0.0. Softmax renormalization cancels most of the sawtooth.
  - out = psum_o[:, 0:128] * (1 / psum_o[:, 128]): reciprocal on DVE,
    the multiply load-balanced between ACT (Copy w/ per-partition scale)
    and DVE; stored per i-chunk so the final DMA is small.

kernel(**inputs) takes FULL unsharded inputs and returns the FULL output.
"""

import numpy as np

B, H, S, D = 2, 16, 2048, 128
N_CORES = 8
HPC = (B * H) // N_CORES  # heads per core = 4
NB = S // 128             # 16 j/i blocks per head
NCH = S // 512            # 4 i-chunks per head
SCALE = 0.08838834764831845
LOG2E = 1.4426950408889634
MASKVAL = -2000.0
EXP_A = float(np.float32(SCALE * 128.0 * LOG2E))
EXP_B = float(np.float32(16256.0 - 7.40))

_CACHED_NC = None


def _build_nc():
    import concourse.bass as bass
    import concourse.mybir as mybir
    import concourse.tile as tile
    from concourse import bacc
    from concourse.masks import make_identity, make_lower_triangular
    from contextlib import ExitStack

    f32 = mybir.dt.float32
    bf16 = mybir.dt.bfloat16
    u16 = mybir.dt.uint16
    Exp = mybir.ActivationFunctionType.Exp
    Copy = mybir.ActivationFunctionType.Copy
    Mult = mybir.AluOpType.mult
    Add = mybir.AluOpType.add

    nc = bacc.Bacc("TRN2", num_devices=N_CORES, debug=False)
    kd = nc.dram_tensor("k", [HPC, S, D], f32, kind="ExternalInput")
    vd = nc.dram_tensor("v", [HPC, S, D], f32, kind="ExternalInput")
    od = nc.dram_tensor("out", [HPC, S, D], f32, kind="ExternalOutput")

    # greedy ACT/DVE load balancing (ns cost model incl. seq overhead)
    eng_ns = {"act": 0.0, "dve": 0.0}

    def exp_costs(fd):
        # ns cost models fit from measured traces
        return (fd + 250) / 1.15, (fd + 120) / 0.96 + 45

    def pick(act_cost, dve_cost):
        if eng_ns["act"] + act_cost <= eng_ns["dve"] + dve_cost:
            eng_ns["act"] += act_cost
            return "act"
        eng_ns["dve"] += dve_cost
        return "dve"

    with tile.TileContext(nc) as tc, ExitStack() as ctx:
        const = ctx.enter_context(tc.tile_pool(name="const", bufs=1))
        loadp = ctx.enter_context(tc.tile_pool(name="load", bufs=2))
        ktp = ctx.enter_context(tc.tile_pool(name="kt", bufs=2))
        vop = ctx.enter_context(tc.tile_pool(name="vop", bufs=2))
        expp = ctx.enter_context(tc.tile_pool(name="expp", bufs=6))
        outp = ctx.enter_context(tc.tile_pool(name="outp", bufs=2))
        smallp = ctx.enter_context(tc.tile_pool(name="small", bufs=8))
        ps_pool = ctx.enter_context(tc.tile_pool(name="ps", bufs=2, space="PSUM"))
        pt_pool = ctx.enter_context(tc.tile_pool(name="pt", bufs=2, space="PSUM"))
        po_pool = ctx.enter_context(tc.tile_pool(name="po", bufs=2, space="PSUM"))

        identbf = const.tile([128, 128], bf16, tag="identbf")
        make_identity(nc, identbf[:, :])
        lowmask_f32 = const.tile([128, 128], f32, tag="lowmask_f32")
        make_lower_triangular(nc, lowmask_f32[:, :], val=MASKVAL, diag=False)
        lowmask = const.tile([128, 128], bf16, tag="lowmask")
        nc.vector.tensor_copy(lowmask[:, :], lowmask_f32[:, :])
        # warmup exp so ACT's one-time table load happens during startup
        warm = const.tile([128, 1], f32, tag="warm")
        nc.scalar.activation(warm[:, :], lowmask_f32[:, 0:1], Exp, scale=SCALE)

        for h in range(HPC):
            # ---- loads: fp32 HBM -> bf16 SBUF (SWDGE cast), natural ----
            knat = loadp.tile([128, NB, 128], bf16, tag="knat")
            vnat = loadp.tile([128, NB, 128], bf16, tag="vnat")
            kview = kd.ap()[h].rearrange("(n p) d -> p n d", p=128)
            vview = vd.ap()[h].rearrange("(n p) d -> p n d", p=128)
            vones = vop.tile([128, NB, 130], bf16, tag="vones")
            KT3 = ktp.tile([128, NB, 128], bf16, tag="KT")
            VT3 = ktp.tile([128, NB, 128], bf16, tag="VT")
            # PE transposes of 4 K and 4 V blocks for one chunk, each
            # matrix landing in ONE 512-wide PSUM tile so a single DVE
            # copy moves it to SBUF (1/4 the DVE instruction count).
            # The 8 transpose-MMs can be dribbled out between pairs of the
            # PREVIOUS chunk as PE filler during its exp waits (tr_step);
            # the two DVE copies stay at the flush point (tr_finish).
            def tr_begin(ci):
                return {
                    "ci": ci,
                    "done": 0,
                    "tiles": [
                        pt_pool.tile(
                            [128, 512], bf16, tag="pt", name=f"pst{nm}_{h}_{ci}"
                        )
                        for nm in "kv"
                    ],
                }

            def tr_step(st, n):
                b0 = 4 * st["ci"]
                while n > 0 and st["done"] < 8:
                    m, u = divmod(st["done"], 4)
                    nc.tensor.transpose(
                        st["tiles"][m][:, u * 128 : (u + 1) * 128],
                        (knat, vnat)[m][:, b0 + u, :],
                        identbf[:, :],
                    )
                    st["done"] += 1
                    n -= 1

            def tr_finish(st):
                tr_step(st, 8)
                b0 = 4 * st["ci"]
                for m, dst in enumerate((KT3, VT3)):
                    nc.vector.tensor_copy(
                        dst[:, b0 : b0 + 4, :], st["tiles"][m][:, :]
                    )
                    eng_ns["dve"] += 420

            def emit_transposes(ci):
                tr_finish(tr_begin(ci))

            # first 4 blocks in their own chunk, with their transposes
            # emitted BEFORE the big load group: a DMA consumer waits on
            # the Q7 drain point at its emission, so this keeps chunk-0's
            # transposes off the big loads' tail
            nc.gpsimd.dma_start(knat[:, 0:4, :], kview[:, 0:4, :])
            nc.gpsimd.dma_start(vnat[:, 0:4, :], vview[:, 0:4, :])
            emit_transposes(0)
            nc.gpsimd.dma_start(knat[:, 4:16, :], kview[:, 4:16, :])
            nc.gpsimd.dma_start(vnat[:, 4:16, :], vview[:, 4:16, :])
            nc.vector.tensor_copy(vones[:, :, 0:128], vnat[:, :, :])
            eng_ns["dve"] += 640
            nc.vector.memset(vones[:, :, 128:130], 1.0)
            eng_ns["dve"] += 110
            KT = KT3.rearrange("p n d -> p (n d)")
            VT = VT3.rearrange("p n d -> p (n d)")

            out_sb = outp.tile([128, NB, 128], f32, tag="out_sb")

            # ---- main causal attention loop ----
            for ci in range(NCH):
                i0b = 4 * ci              # first i-block of chunk
                iend = (i0b + 4) * 128
                po = [
                    po_pool.tile([128, 258], f32, tag="po", name=f"po_{h}_{ci}_{u}")
                    for u in range(2)
                ]

                def po_ap(bi):
                    u = bi - i0b
                    return po[u // 2][:, (u % 2) * 129 : (u % 2) * 129 + 129]

                # pairs emitted with one-pair lookahead: pair k+1's score
                # matmuls + exp come before pair k's MM2s, so the PE has
                # work while the first MM2 of a chunk waits for po banks
                pending = None  # (bj_pair_state, ex) awaiting MM2 emission
                pairs = list(range(0, i0b + 4, 2)) + [None]
                for bja in pairs:
                    cur = None
                    if bja is None and ci + 1 < NCH:
                        # next chunk's transposes go here: PE slots them
                        # between this chunk's MM1s and final MM2s; the DVE
                        # copies queue BEHIND this chunk's exps, not ahead
                        emit_transposes(ci + 1)
                    if bja is not None:
                        bjb = bja + 1
                        ista = max(i0b, bja) * 128
                        istb_ = max(i0b, bjb) * 128
                        n1a = iend - ista
                        n1b = iend - istb_
                        fd = n1a + n1b
                        ps = ps_pool.tile([128, 1024], f32, tag="ps")
                        # bank of region B: 0 if it fits below col 512
                        same_bank = (n1a + n1b) <= 512
                        diag_a = bja >= i0b
                        diag_b = bjb >= i0b
                        # bank A writers: mm1a (+ maskA); bank B writers:
                        # mm1b (+ maskB); same_bank merges the groups
                        a_stop = (not diag_a) and not same_bank
                        nc.tensor.matmul(
                            ps[:, 0:n1a],
                            VT[:, bja * 128 : (bja + 1) * 128],
                            KT[:, ista:iend],
                            start=True,
                            stop=a_stop,
                            skip_group_check=True,
                        )
                        if diag_a:
                            nc.tensor.matmul(
                                ps[:, 0:128],
                                identbf[:, :],
                                lowmask[:, :],
                                start=False,
                                stop=not same_bank,
                                skip_group_check=True,
                            )
                        nc.tensor.matmul(
                            ps[:, n1a : n1a + n1b],
                            VT[:, bjb * 128 : (bjb + 1) * 128],
                            KT[:, istb_:iend],
                            start=not same_bank,
                            stop=not diag_b,
                            skip_group_check=True,
                        )
                        if diag_b:
                            nc.tensor.matmul(
                                ps[:, n1a : n1a + 128],
                                identbf[:, :],
                                lowmask[:, :],
                                start=False,
                                stop=True,
                                skip_group_check=True,
                            )
                        ex = expp.tile([128, 1024], bf16, tag="ex")
                        ca, cd = exp_costs(fd)
                        if pick(ca, cd) == "act":
                            nc.scalar.activation(
                                ex[:, 0:fd], ps[:, 0:fd], Exp, scale=SCALE
                            )
                        else:
                            nc.vector.tensor_scalar(
                                ex[:, 0:fd].bitcast(u16),
                                ps[:, 0:fd],
                                EXP_A,
                                EXP_B,
                                Mult,
                                Add,
                            )
                        cur = ((bja, ista, 0), (bjb, istb_, n1a), ex)
                    if pending is not None:
                        (pa, pb, pex) = pending
                        for bj, ist, off in (pa, pb):
                            for bi in range(ist // 128, i0b + 4):
                                c0 = off + bi * 128 - ist
                                nc.tensor.matmul(
                                    po_ap(bi),
                                    pex[:, c0 : c0 + 128],
                                    vones[:, bj, 0:129],
                                    start=(bj == 0 and (bi - i0b) % 2 == 0),
                                    stop=(bj == bi and (bi - i0b) % 2 == 1),
                                    skip_group_check=True,
                                )
                    pending = cur
                # epilogue: 2 strided recips, then the 4 per-block
                # normalizing multiplies load-balanced ACT/DVE
                rcs = []
                for t in range(2):
                    rc = smallp.tile([128, 2], f32, tag="rc")
                    den = po[t].rearrange("p (u c) -> p u c", c=129)[:, :, 128]
                    nc.vector.reciprocal(rc[:, :], den)
                    eng_ns["dve"] += 190
                    rcs.append(rc)
                for u in range(4):
                    bi = i0b + u
                    rc = rcs[u // 2][:, (u % 2) : (u % 2) + 1]
                    if pick(430.0, 290.0) == "act":
                        nc.scalar.activation(
                            out_sb[:, bi, :],
                            po_ap(bi)[:, 0:128],
                            Copy,
                            scale=rc,
                        )
                    else:
                        nc.vector.tensor_scalar_mul(
                            out_sb[:, bi, :], po_ap(bi)[:, 0:128], rc
                        )
                nc.sync.dma_start(
                    od.ap()[h].rearrange("(n p) d -> p n d", p=128)[
                        :, i0b : i0b + 4, :
                    ],
                    out_sb[:, i0b : i0b + 4, :],
                )

    nc.finalize()
    return nc


def _get_nc():
    global _CACHED_NC
    if _CACHED_NC is None:
        _CACHED_NC = _build_nc()
    return _CACHED_NC


def run_sharded(k, v, trace=False):
    """k, v: [B*H, S, D] fp32. Returns (out [B*H, S, D], BassKernelResults)."""
    from concourse import bass_utils

    nc = _get_nc()
    in_maps = [
        {
            "k": np.ascontiguousarray(k[c * HPC : (c + 1) * HPC]),
            "v": np.ascontiguousarray(v[c * HPC : (c + 1) * HPC]),
        }
        for c in range(N_CORES)
    ]
    res = bass_utils.run_bass_kernel_spmd(
        nc, in_maps, core_ids=list(range(N_CORES)), trace=trace
    )
    out = np.concatenate([res.results[c]["out"] for c in range(N_CORES)], axis=0)
    return out, res


def kernel(q, k, v):
    k = np.asarray(k, dtype=np.float32).reshape(B * H, S, D)
    v = np.asarray(v, dtype=np.float32).reshape(B * H, S, D)
    out, _ = run_sharded(k, v, trace=False)
    return out.reshape(B, H, S, D)

